# revision 1
# baseline (speedup 1.0000x reference)
"""Behler-Parrinello NN energy kernel for 8 Trainium2 NeuronCores.

Strategy
--------
Data-parallel over atoms (125k H + 125k O per core). Host-side (numpy):
  * assigns every molecule (per core, per element) to one of 128 SBUF
    partitions with a count-balanced snake schedule,
  * lays the core's atoms out in a [128 partitions x 992 columns] slot
    grid per element (padded ~1.6%), so the device-side MLP emits each
    atom's energy directly into its partition,
  * pre-computes per-atom bin indices so a gpsimd `local_scatter` +
    vector add performs the segment reduction entirely on-device,
  * transposes + casts features to fp16 feature-major layout for
    fully-contiguous DMA.

Device-side per core (Bass/Tile):
  * stream X^T tiles (fp16), W1 matmuls col-packed 2-up and W2 as a
    single block-diagonal full-row matmul (4-up partition packing) so
    tanh activations run 128 lanes wide; only two distinct PE row
    configs are ever in flight (>=3 concurrent row-group configs
    crashes TRN2 execution),
  * third (linear) layer is fused with the energy transpose: a matmul
    with lhsT = h2 4-pack slice and rhs = 4-block diag(W3) yields
    psum[atom, 32*(2sb+half)+k] = h2[k, atom] * W3[k]; a segmented
    vector reduce with a permuted output AP produces fp16 energy
    columns E[partition, slot-column],
  * gpsimd local_scatter batches place energies into per-partition bins
    (host plan guarantees no duplicates per batch), vector adds
    accumulate into a fp32 [128, 1280] table,
  * the bias b3 contribution (count * b3) and the bin->molecule merge
    happen on host, plus the 8-core sum.
"""

import sys

if "/opt/trn_rl_repo" not in sys.path:
    sys.path.insert(0, "/opt/trn_rl_repo")

import numpy as np

# ---------------------------------------------------------------- constants
N_CORES = 8
N_MOL = 100_000
N_FEAT = 128
NH1 = 64
NH2 = 32
N_ATOMS = 1_000_000          # per element, global
APC = N_ATOMS // N_CORES     # atoms per core per element (125000)

T_COLS = 992                 # slot columns per partition per element
NB = 4                       # scatter batches per element
BW = T_COLS // NB            # columns per batch (256)
SLOTS = 128 * T_COLS         # slots per core per element (131072)
N_BINS = 1280                # bins per partition (H: [0,640), O: [640,1280))
BIN_HALF = 640
MACRO = 16                   # slot-columns per macro-block (2048 atoms)
N_MACRO = T_COLS // MACRO    # macro blocks per element (64)
DMA_COLS = 4096              # slots per macro pair
XCHUNK = 8192                # slots per feature DMA (2 MB)

_CACHE = {}


# ================================================================ device IR
def _build_nc():
    import concourse.bacc as bacc
    import concourse.mybir as mybir
    from concourse.tile import TileContext

    dt = mybir.dt
    f16, f32, i16 = dt.float16, dt.float32, dt.int16
    Tanh = mybir.ActivationFunctionType.Tanh

    nc = bacc.Bacc("TRN2", target_bir_lowering=False, debug=False)

    xt = {
        e: nc.dram_tensor(f"xt_{e}", [128, SLOTS], f16, kind="ExternalInput")
        for e in ("h", "o")
    }
    wpk = {
        e: nc.dram_tensor(f"wpk_{e}", [128, 256], f16, kind="ExternalInput")
        for e in ("h", "o")
    }
    bpk = {
        e: nc.dram_tensor(f"bpk_{e}", [128, 2], f32, kind="ExternalInput")
        for e in ("h", "o")
    }
    q_idx = nc.dram_tensor("q_idx", [128, 2 * T_COLS], i16, kind="ExternalInput")
    out_acc = nc.dram_tensor("out_acc", [128, N_BINS], f32, kind="ExternalOutput")

    with TileContext(nc) as tc:
        with (
            tc.tile_pool(name="wpool", bufs=1) as wpool,
            tc.tile_pool(name="xpool", bufs=5) as xpool,
            tc.tile_pool(name="hpool", bufs=3) as hpool,
            tc.tile_pool(name="epool", bufs=1) as epool,
            tc.tile_pool(name="spool", bufs=2) as spool,
            tc.tile_pool(name="ps1", bufs=2, space="PSUM") as ps1,
            tc.tile_pool(name="ps2", bufs=1, space="PSUM") as ps2,
            tc.tile_pool(name="ps3", bufs=2, space="PSUM") as ps3,
        ):
            # --- persistent tiles
            E = epool.tile([128, 2 * T_COLS], f16, tag="E")
            Q = epool.tile([128, 2 * T_COLS], i16, tag="Q")
            acc = epool.tile([128, N_BINS], f32, tag="acc")
            nc.vector.memset(acc[:], 0.0)

            warm = epool.tile([128, 1], f32, tag="warm")
            nc.scalar.activation(
                warm[:], acc[:, 0:1], mybir.ActivationFunctionType.Tanh
            )

            wt = {}
            for e in ("h", "o"):
                wtile = wpool.tile([128, 256], f16, tag=f"wp{e}", name=f"wp{e}")
                btile = wpool.tile([128, 2], f32, tag=f"bp{e}", name=f"bp{e}")
                nc.sync.dma_start(wtile[:], wpk[e][:])
                nc.sync.dma_start(btile[:], bpk[e][:])
                wt[e] = {
                    "w1": wtile[:, 0:64],
                    "w2s": wtile[:, 64:128],
                    "w3d": wtile[:, 128:256],
                    "b1s": btile[:, 0:1],
                    "b2s": btile[:, 1:2],
                }

            for ei, e in enumerate(("h", "o")):
                W = wt[e]
                xtiles = {}
                # macro block = 2 super-blocks = 2048 atoms = 16 slot cols
                for jp in range(N_MACRO // 2):      # macro pair: 4096 atoms
                    # one 1 MB DMA per macro pair; the very first tile is
                    # split in halves for fast pipeline fill
                    di = jp
                    xtiles[di] = xpool.tile([128, DMA_COLS], f16, tag="xt", name=f"xt{e}{di}")
                    nc.sync.dma_start(
                        xtiles[di][:],
                        xt[e][:, di * DMA_COLS : (di + 1) * DMA_COLS],
                    )
                    xtile = xtiles[di]
                    h1s = []
                    for j2 in range(2):
                        # layer 1: 4 full-row matmuls (2 super-blocks,
                        # col-packed pairs) into a 2-bank psum tile
                        p1 = ps1.tile([128, 1024], f32, tag="p1", name=f"p1_{e}{jp}_{j2}")
                        for sb in range(2):
                            for blk in range(2):
                                o = 2048 * j2 + 1024 * sb + 512 * blk
                                nc.tensor.matmul(
                                    p1[64 * blk : 64 * blk + 64,
                                       512 * sb : 512 * (sb + 1)],
                                    W["w1"],
                                    xtile[:, o : o + 512],
                                    tile_position=(0, 64 * blk),
                                )
                        # tanh -> h1 (fp16), 128 lanes x 1024
                        h1 = hpool.tile([128, 1024], f16, tag="h1", name=f"h1_{e}{jp}_{j2}")
                        nc.scalar.activation(h1[:], p1[:], Tanh, bias=W["b1s"])
                        h1s.append(h1)
                    # layer 2: 4 matmuls with block-diag W2 into a 2-bank
                    # psum tile; macro m -> cols 512*m, sb0 rows 0:64 (tp
                    # (0,0)), sb1 rows 64:128 (tp (0,64))
                    p2 = ps2.tile([128, 1024], f32, tag="p2", name=f"p2_{e}{jp}")
                    for j2 in range(2):
                        for sb in range(2):
                            nc.tensor.matmul(
                                p2[64 * sb : 64 * sb + 64,
                                   512 * j2 : 512 * (j2 + 1)],
                                W["w2s"],
                                h1s[j2][:, 512 * sb : 512 * (sb + 1)],
                                tile_position=(0, 64 * sb),
                            )
                    # tanh -> h2 (fp16): row r = 64*sb + 32*half
                    h2 = hpool.tile([128, 1024], f16, tag="h2", name=f"h2_{e}{jp}")
                    nc.scalar.activation(h2[:], p2[:], Tanh, bias=W["b2s"])
                    # layer 3 fused with transpose: lhsT = h2 4-pack slice,
                    # rhs = 4-block diag(W3): psum3[atom, 32*(2sb+half)+k]
                    for j2 in range(2):
                        j = 2 * jp + j2
                        p3 = ps3.tile([128, 512], f32, tag="p3", name=f"p3_{e}{j}")
                        for q in range(4):
                            nc.tensor.matmul(
                                p3[:, 128 * q : 128 * (q + 1)],
                                h2[:, 512 * j2 + 128 * q : 512 * j2 + 128 * (q + 1)],
                                W["w3d"],
                                tile_position=(0, 0),
                            )
                        # segmented reduce -> fp16 energy columns (permuted)
                        # psum col = 128q + 32*(2sb+half) + k ; E col offset
                        # = 8sb + 4half + q
                        ecol = ei * T_COLS + j * MACRO
                        with nc.allow_low_precision("fp16 energies, fp32 accum"):
                            nc.vector.tensor_reduce(
                                E[:, ecol : ecol + MACRO].rearrange(
                                    "p (sb half q) -> p q sb half",
                                    sb=2, half=2, q=4,
                                ),
                                p3[:].rearrange(
                                    "p (q sb half k) -> p q sb half k",
                                    q=4, sb=2, half=2, k=NH2,
                                ),
                                axis=mybir.AxisListType.X,
                                op=mybir.AluOpType.add,
                            )

            # ---- scatter batches: bins <- energies, accumulate fp32
            # (the very last batch is split so the tail only waits on the
            # final macro's 8 columns)
            nc.sync.dma_start(Q[:], q_idx[:])
            ranges = [(bi * BW, BW) for bi in range(2 * NB - 1)]
            last = (2 * NB - 1) * BW
            ranges += [(last, BW - 8), (last + BW - 8, 8)]
            for bi, (r0, w) in enumerate(ranges):
                S = spool.tile([128, N_BINS], f16, tag="S", name=f"S{bi}")
                nc.gpsimd.local_scatter(
                    S[:],
                    E[:, r0 : r0 + w],
                    Q[:, r0 : r0 + w],
                    channels=128,
                    num_elems=N_BINS,
                    num_idxs=w,
                )
                nc.vector.tensor_tensor(
                    acc[:], acc[:], S[:], op=mybir.AluOpType.add
                )

            nc.sync.dma_start(out_acc[:], acc[:])

    nc.compile()
    return nc


# ================================================================ host plan
def _plan_element(m, rng_unused=None):
    """Plan one (core, element): molecule->partition, atom->slot, bins.

    m: int32 [n] molecule index per atom (core's shard).
    Returns (perm, q, bin_mol, bin_p, bin_id) where
      perm   int64 [SLOTS] source atom per slot (pads -> 0),
      q      int16 [128, T_COLS] bin per slot (-1 for pads),
      bin_mol/bin_p/bin_id: molecule ids and (partition, bin) locations
      for the host-side merge.
    """
    n = m.shape[0]
    cnt = np.bincount(m, minlength=N_MOL)
    present = np.flatnonzero(cnt)
    # snake assignment of present molecules (count-desc) to partitions
    order = present[np.argsort(-cnt[present], kind="stable")]
    r = np.arange(order.size)
    pat = r % 256
    p_of_rank = np.where(pat < 128, pat, 255 - pat)
    p_assign = np.full(N_MOL, -1, np.int32)
    p_assign[order] = p_of_rank
    # primary bin = rank of molecule within its partition (by snake order)
    prim = np.full(N_MOL, -1, np.int32)
    # stable sort ranks by partition, preserving snake order inside
    o2 = np.argsort(p_of_rank, kind="stable")
    pp = p_of_rank[o2]
    starts = np.searchsorted(pp, np.arange(128))
    within = np.arange(order.size) - starts[pp]
    prim[order[o2]] = within
    n_prim = np.bincount(pp, minlength=128)

    # per-atom occurrence rank k within its molecule
    a_sort = np.argsort(m, kind="stable")
    ms = m[a_sort]
    gstart = np.r_[0, np.flatnonzero(np.diff(ms)) + 1]
    glen = np.diff(np.r_[gstart, n])
    k = np.arange(n) - np.repeat(gstart, glen)
    level = k // NB

    # spill bins for occurrences beyond NB per molecule
    bins_sorted = prim[ms].copy()
    sp_first = (level >= 1) & (k % NB == 0)
    if sp_first.any():
        sp_pos = np.flatnonzero(sp_first)
        sp_p = p_assign[ms[sp_pos]]
        so = np.argsort(sp_p, kind="stable")
        sp_sorted_p = sp_p[so]
        sp_starts = np.searchsorted(sp_sorted_p, np.arange(128))
        sp_within = np.arange(sp_pos.size) - sp_starts[sp_sorted_p]
        sp_bin = np.empty(sp_pos.size, np.int32)
        sp_bin[so] = n_prim[sp_sorted_p] + sp_within
        # propagate each spill group's bin to its (<= NB) member atoms
        gid = np.cumsum(sp_first) - 1
        lvl_mask = level >= 1
        bins_sorted[lvl_mask] = sp_bin[gid[lvl_mask]]
        sp_mol = ms[sp_pos]
        sp_part = p_assign[sp_mol]
    else:
        sp_bin = np.empty(0, np.int32)
        sp_mol = np.empty(0, np.int32)
        sp_part = np.empty(0, np.int32)

    p_atom = p_assign[ms]
    # position within partition: sort by (partition, bin, k)
    o3 = np.lexsort((k, bins_sorted, p_atom))
    p3 = p_atom[o3]
    pstarts = np.searchsorted(p3, np.arange(128))
    pos = np.arange(n) - pstarts[p3]
    load = np.bincount(p3, minlength=128)
    if load.max() > T_COLS:
        raise RuntimeError(f"partition overload {load.max()} > {T_COLS}")
    nb_used = int(n_prim.max() + (np.bincount(sp_part, minlength=128).max()
                                  if sp_part.size else 0))
    if nb_used > BIN_HALF:
        raise RuntimeError(f"bins overload {nb_used} > {BIN_HALF}")

    batch = pos % NB
    col = batch * BW + pos // NB
    atom_ids = a_sort[o3]

    perm = np.zeros(SLOTS, np.int64)
    q = np.full((128, T_COLS), -1, np.int16)
    slot = col * 128 + p3
    perm[slot] = atom_ids
    q[p3, col] = bins_sorted[o3]

    bin_mol = np.concatenate([order, sp_mol])
    bin_p = np.concatenate([p_of_rank, sp_part])
    bin_id = np.concatenate([prim[order], sp_bin])
    return perm, q, bin_mol, bin_p, bin_id


def _prep_weights(W1, b1, W2, b2, W3):
    w1 = np.ascontiguousarray(W1, np.float16)                       # [128, 64]
    w2s = np.zeros((128, NH1), np.float32)                          # block-diag
    w2s[0:64, 0:32] = W2
    w2s[64:128, 32:64] = W2
    w2s = np.ascontiguousarray(w2s, np.float16)
    dg = np.diag(np.asarray(W3)[:, 0])
    w3d = np.zeros((128, 128), np.float32)                          # 4-block diag
    for c in range(4):
        w3d[32 * c : 32 * c + 32, 32 * c : 32 * c + 32] = dg
    w3d = np.ascontiguousarray(w3d, np.float16)
    b1c = np.asarray(b1, np.float32).reshape(-1, 1)
    b2c = np.asarray(b2, np.float32).reshape(-1, 1)
    b1s = np.vstack([b1c, b1c]).astype(np.float32)                  # [128, 1]
    b2s = np.vstack([b2c] * 4).astype(np.float32)                   # [128, 1]
    wpk = np.ascontiguousarray(np.hstack([w1, w2s, w3d]), np.float16)
    bpk = np.ascontiguousarray(np.hstack([b1s, b2s]), np.float32)
    return wpk, bpk


# ================================================================ entry
def _prepare(
    feats_H, feats_O, mol_idx_H, mol_idx_O,
    W1_H, b1_H, W2_H, b2_H, W3_H,
    W1_O, b1_O, W2_O, b2_O, W3_O,
):
    feats = {"h": np.asarray(feats_H), "o": np.asarray(feats_O)}
    mols = {
        "h": np.asarray(mol_idx_H, np.int32),
        "o": np.asarray(mol_idx_O, np.int32),
    }
    wts = {
        "h": _prep_weights(W1_H, b1_H, W2_H, b2_H, W3_H),
        "o": _prep_weights(W1_O, b1_O, W2_O, b2_O, W3_O),
    }

    in_maps = []
    merge = []            # per core: (bin_mol, bin_p, bin_id, elem)
    for c in range(N_CORES):
        im = {}
        mg = []
        q_full = np.empty((128, 2 * T_COLS), np.int16)
        for ei, e in enumerate(("h", "o")):
            sl = slice(c * APC, (c + 1) * APC)
            perm, q, bm, bp, bid = _plan_element(mols[e][sl])
            xs = feats[e][sl]
            xtp = np.ascontiguousarray(
                xs.astype(np.float16)[perm].T
            )                                            # [128, SLOTS]
            im[f"xt_{e}"] = xtp
            off = ei * BIN_HALF
            q_full[:, ei * T_COLS : (ei + 1) * T_COLS] = np.where(
                q >= 0, q + off, -1
            )
            mg.append((bm, bp, bid + off))
            im[f"wpk_{e}"], im[f"bpk_{e}"] = wts[e]
        im["q_idx"] = q_full
        in_maps.append(im)
        merge.append(mg)
    return in_maps, merge


def kernel(
    feats_H, feats_O, mol_idx_H, mol_idx_O, n_molecules,
    W1_H, b1_H, W2_H, b2_H, W3_H, b3_H,
    W1_O, b1_O, W2_O, b2_O, W3_O, b3_O,
):
    from concourse import bass_utils

    in_maps, merge = _prepare(
        feats_H, feats_O, mol_idx_H, mol_idx_O,
        W1_H, b1_H, W2_H, b2_H, W3_H,
        W1_O, b1_O, W2_O, b2_O, W3_O,
    )
    if "nc" not in _CACHE:
        _CACHE["nc"] = _build_nc()
    nc = _CACHE["nc"]

    _CACHE["in_maps"] = in_maps
    res = bass_utils.run_bass_kernel_spmd(
        nc, in_maps, core_ids=list(range(N_CORES))
    )

    mols = {
        "h": np.asarray(mol_idx_H, np.int32),
        "o": np.asarray(mol_idx_O, np.int32),
    }
    out = np.zeros(N_MOL, np.float64)
    for c in range(N_CORES):
        acc = res.results[c]["out_acc"]
        for bm, bp, bid in merge[c]:
            out += np.bincount(
                bm, weights=acc[bp, bid].astype(np.float64), minlength=N_MOL
            )
    cnt_h = np.bincount(mols["h"], minlength=N_MOL)
    cnt_o = np.bincount(mols["o"], minlength=N_MOL)
    out += cnt_h * float(np.asarray(b3_H).reshape(()))
    out += cnt_o * float(np.asarray(b3_O).reshape(()))
    return out.astype(np.float32)

